# revision 29
# baseline (speedup 1.0000x reference)
"""Trainium2 Bass kernel for nn_Eq_NLMP_18013092840057 (gnn_message_passing).

Strategy:
  * Host: sort edges by dst; shard into 8 contiguous node ranges (1280
    nodes/core) so no cross-core reduction is needed; group each core's
    edges into 10 windows of 128 destination nodes; pad every window to a
    uniform tile count (T_w tiles of 128 edges, dummy edges have norm=0).
    Host also precomputes the tiny radial-MLP hidden layers h1/h2 (emb @
    fc_w1, 10->16) and pre-scales/permutes the fc_w2 matrices.
  * Device (per core): PE generates the per-edge tensor-product weights
    (h1/h2 [16] x fc_w2 [16,896] per tile), indirect DMA gathers
    x[src]/x[dst], DVE/ACT evaluate the equivariant tensor products and
    gating, and an accumulating one-hot matmul in PSUM performs the
    per-window segment sum.  Window results stream out with plain DMAs.
"""
import sys
import numpy as np

try:
    import concourse.bass as bass  # noqa: F401
except Exception:  # pragma: no cover
    sys.path.insert(0, "/opt/trn_rl_repo")

import concourse.bass as bass
import concourse.bacc as bacc
import concourse.tile as tile
from concourse import mybir
from concourse.bass_utils import run_bass_kernel_spmd

SQRT3 = 3.0 ** 0.5
P = 128
NCORES = 8
dt = mybir.dt
Alu = mybir.AluOpType
Act = mybir.ActivationFunctionType

_KERNEL_CACHE = {}


# --------------------------------------------------------------------------
# Host-side preparation
# --------------------------------------------------------------------------

def _host_prep(x, edge_src, edge_dst, edge_vec, emb, norm,
               fc1_w1, fc1_w2, fc2_w1, fc2_w2):
    N = x.shape[0]
    E = edge_src.shape[0]
    npc = ((N + NCORES * P - 1) // (NCORES * P)) * P          # nodes per core
    wpc = npc // P                                             # windows/core

    order = np.argsort(edge_dst, kind="stable")
    dst_s = edge_dst[order]
    win = (dst_s // P).astype(np.int64)
    n_windows = NCORES * wpc
    counts = np.bincount(win, minlength=n_windows)
    t_w = int(max(1, ((counts + P - 1) // P).max()))
    nt = t_w * wpc                                             # tiles per core
    ep = nt * P                                                # padded edges/core

    # padded per-window edge-id table
    idx_pad = np.full((n_windows, t_w * P), -1, np.int64)
    starts = np.concatenate([[0], np.cumsum(counts)])
    for w in range(n_windows):
        c = counts[w]
        idx_pad[w, :c] = order[starts[w]:starts[w] + c]

    # host hidden layers (10 -> 16), relu
    h1 = np.maximum(emb @ fc1_w1 / np.sqrt(np.float32(10.0)), 0.0).astype(np.float32)
    h2 = np.maximum(emb @ fc2_w1 / np.sqrt(np.float32(10.0)), 0.0).astype(np.float32)

    # permuted / pre-scaled fc2 weights  [16, 896]
    s = np.float32(1.0 / np.sqrt(16.0))
    a1 = np.float32(1.0 / np.sqrt(32.0))
    a2 = np.float32(1.0 / np.sqrt(16.0))
    f1 = (fc1_w2 * (s * a1)).astype(np.float32)
    ss, vv, sv, vs = f1[:, 0:128], f1[:, 128:256], f1[:, 256:384], f1[:, 384:512]
    fc1p = np.concatenate([ss, sv * np.float32(SQRT3), vv, vs], axis=1)
    f2 = (fc2_w2 * (s * a2)).astype(np.float32)
    Ass, Avv = f2[:, 0:64], f2[:, 64:128]
    Bss, Bvv = f2[:, 128:192], f2[:, 192:256]
    Csv, Cvs = f2[:, 256:320], f2[:, 320:384]
    fc2p = np.concatenate([Ass, Bss, Csv * np.float32(SQRT3), Avv, Bvv, Cvs], axis=1)
    fcw = np.concatenate([fc1p, fc2p], axis=1)                 # [16, 896]

    def interleave(arr):
        # [ep, F] -> [128, nt*F]  with edge (t,p) at [p, t*F:(t+1)*F]
        F = arr.shape[1]
        return np.ascontiguousarray(
            arr.reshape(nt, P, F).transpose(1, 0, 2).reshape(P, nt * F))

    in_maps = []
    for m in range(NCORES):
        ids = idx_pad[m * wpc:(m + 1) * wpc].reshape(-1)       # [ep]
        valid = ids >= 0
        idc = np.where(valid, ids, 0)
        vm = valid[:, None]

        vec_c = np.where(vm, edge_vec[idc], np.float32(1.0)).astype(np.float32)
        norm_c = np.where(valid, norm[idc], 0.0).astype(np.float32)[:, None]
        src_c = np.where(valid, edge_src[idc], 0).astype(np.int32)[:, None]
        dstg_c = np.where(valid, edge_dst[idc], 0).astype(np.int32)[:, None]
        dstf_c = np.where(valid, edge_dst[idc] % P, 0).astype(np.float32)[:, None]
        h1_c = np.where(vm, h1[idc], 0.0).astype(np.float32)   # [ep,16]
        h2_c = np.where(vm, h2[idc], 0.0).astype(np.float32)

        in_maps.append({
            "x": np.ascontiguousarray(x.astype(np.float32)),
            "fcw": np.ascontiguousarray(fcw),
            "h1t": np.ascontiguousarray(h1_c.T),               # [16, ep]
            "h2t": np.ascontiguousarray(h2_c.T),
            "vecil": interleave(vec_c),
            "normil": interleave(norm_c),
            "srcil": interleave(src_c),
            "dstgil": interleave(dstg_c),
            "dstfil": interleave(dstf_c),
        })
    return in_maps, N, npc, wpc, t_w, nt, ep


# --------------------------------------------------------------------------
# Bass program
# --------------------------------------------------------------------------

def _build(N, npc, wpc, t_w, nt, ep):
    nc = bacc.Bacc("TRN2", target_bir_lowering=False)
    f32 = dt.float32

    x_d = nc.dram_tensor("x", [N, 32], f32, kind="ExternalInput")
    fcw_d = nc.dram_tensor("fcw", [16, 896], f32, kind="ExternalInput")
    h1t_d = nc.dram_tensor("h1t", [16, ep], f32, kind="ExternalInput")
    h2t_d = nc.dram_tensor("h2t", [16, ep], f32, kind="ExternalInput")
    vec_d = nc.dram_tensor("vecil", [P, nt * 3], f32, kind="ExternalInput")
    norm_d = nc.dram_tensor("normil", [P, nt], f32, kind="ExternalInput")
    src_d = nc.dram_tensor("srcil", [P, nt], dt.int32, kind="ExternalInput")
    dstg_d = nc.dram_tensor("dstgil", [P, nt], dt.int32, kind="ExternalInput")
    dstf_d = nc.dram_tensor("dstfil", [P, nt], f32, kind="ExternalInput")
    out_d = nc.dram_tensor("out", [npc, 32], f32, kind="ExternalOutput")

    with tile.TileContext(nc) as tc:
        with tc.tile_pool(name="const", bufs=1) as cpool, \
             tc.tile_pool(name="io", bufs=2) as io, \
             tc.tile_pool(name="big", bufs=1) as big, \
             tc.tile_pool(name="sm", bufs=1) as sm, \
             tc.tile_pool(name="oh", bufs=1) as ohp, \
             tc.tile_pool(name="wps", bufs=3, space="PSUM") as wps, \
             tc.tile_pool(name="aps", bufs=2, space="PSUM") as aps:

            fcw = cpool.tile([16, 896], f32)
            nc.sync.dma_start(fcw[:], fcw_d[:, :])
            iota_i = cpool.tile([P, P], dt.int32)
            iota_f = cpool.tile([P, P], f32)
            nc.gpsimd.iota(iota_i[:], pattern=[[1, P]], base=0, channel_multiplier=0)
            nc.vector.tensor_copy(iota_f[:], iota_i[:])

            # geometry prologue: r = vec/|vec| for every tile at once (one
            # Sqrt table residency; windows below then only need Tanh)
            vec_all = cpool.tile([P, nt, 3], f32)
            nc.sync.dma_start(vec_all[:], vec_d[:, :].rearrange("p (t k) -> p t k", k=3))
            rsq_a = sm.tile([P, nt, 3], f32, tag="tmpE")
            ssum_a = cpool.tile([P, nt], f32)
            rq_a = cpool.tile([P, nt], f32)
            nc.vector.tensor_tensor(out=rsq_a[:], in0=vec_all[:], in1=vec_all[:], op=Alu.mult)
            nc.vector.tensor_reduce(out=ssum_a[:], in_=rsq_a[:],
                                    axis=mybir.AxisListType.X, op=Alu.add)
            nc.scalar.activation(ssum_a[:], ssum_a[:], Act.Sqrt)
            nc.vector.reciprocal(rq_a[:], ssum_a[:])
            nc.vector.tensor_tensor(out=vec_all[:], in0=vec_all[:],
                                    in1=rq_a[:].unsqueeze(2).broadcast_to([P, nt, 3]),
                                    op=Alu.mult)

            for w in range(wpc):
                tb = w * t_w            # tile base
                eb = tb * P             # edge base

                h1c = io.tile([16, t_w * P], f32, tag="h1c")
                h2c = io.tile([16, t_w * P], f32, tag="h2c")
                nrm = io.tile([P, t_w], f32, tag="nrm")
                srci = io.tile([P, t_w], dt.int32, tag="srci")
                dstgi = io.tile([P, t_w], dt.int32, tag="dstgi")
                dstf = io.tile([P, t_w], f32, tag="dstf")
                nc.sync.dma_start(h1c[:], h1t_d[:, eb:eb + t_w * P])
                nc.sync.dma_start(h2c[:], h2t_d[:, eb:eb + t_w * P])
                nc.sync.dma_start(nrm[:], norm_d[:, tb:tb + t_w])
                nc.sync.dma_start(srci[:], src_d[:, tb:tb + t_w])
                nc.sync.dma_start(dstgi[:], dstg_d[:, tb:tb + t_w])
                nc.sync.dma_start(dstf[:], dstf_d[:, tb:tb + t_w])

                xs = io.tile([P, t_w, 32], f32, tag="xs")
                xd = io.tile([P, t_w, 32], f32, tag="xd")
                W1 = big.tile([P, t_w, 512], f32, tag="W1")
                W2 = big.tile([P, t_w, 384], f32, tag="W2")

                for t in range(t_w):
                    nc.gpsimd.indirect_dma_start(
                        out=xs[:, t, :], out_offset=None, in_=x_d[:, :],
                        in_offset=bass.IndirectOffsetOnAxis(ap=srci[:, t:t + 1], axis=0))
                    nc.gpsimd.indirect_dma_start(
                        out=xd[:, t, :], out_offset=None, in_=x_d[:, :],
                        in_offset=bass.IndirectOffsetOnAxis(ap=dstgi[:, t:t + 1], axis=0))
                    wp = wps.tile([P, 1024], f32, tag="wp")
                    nc.tensor.matmul(out=wp[:, 0:512],
                                     lhsT=h1c[:, t * P:(t + 1) * P],
                                     rhs=fcw[:, 0:512], start=True, stop=True)
                    nc.tensor.matmul(out=wp[:, 512:896],
                                     lhsT=h2c[:, t * P:(t + 1) * P],
                                     rhs=fcw[:, 512:896], start=True, stop=True)
                    nc.scalar.copy(W1[:, t, :], wp[:, 0:512])
                    nc.scalar.copy(W2[:, t, :], wp[:, 512:896])

                r = vec_all[:, tb:tb + t_w, :]

                # ---- vdot1[u] = sum_k v1[u,k] r[k] ---------------------------
                tmpE = sm.tile([P, t_w, 16, 3], f32, tag="tmpE")
                vdot1 = sm.tile([P, t_w, 16], f32, tag="vdot1")
                rb8 = r[:].unsqueeze(2).broadcast_to([P, t_w, 8, 3])
                xsv = xs[:, :, 8:32].rearrange("p t (u k) -> p t u k", u=8, k=3)
                xdv = xd[:, :, 8:32].rearrange("p t (u k) -> p t u k", u=8, k=3)
                nc.vector.tensor_tensor(out=tmpE[:, :, 0:8, :], in0=xsv, in1=rb8, op=Alu.mult)
                nc.vector.tensor_tensor(out=tmpE[:, :, 8:16, :], in0=xdv, in1=rb8, op=Alu.mult)
                nc.vector.tensor_reduce(out=vdot1[:], in_=tmpE[:],
                                        axis=mybir.AxisListType.X, op=Alu.add)

                # ---- TP1 scalar paths: tmpA [p,t,3,16,8] ---------------------
                # (DVE ISA allows at most 3 free dims per AP: split by path,
                #  merge contiguous (u,w) for the reduction tree.)
                tmpA = big.tile([P, t_w, 3, 16, 8], f32, tag="tmpA")
                s1s = xs[:, :, 0:8].unsqueeze(3).broadcast_to([P, t_w, 8, 8])
                s1d = xd[:, :, 0:8].unsqueeze(3).broadcast_to([P, t_w, 8, 8])
                for q in (0, 1):   # 0: Wss, 1: Wsv
                    Wq = W1[:, :, q * 128:(q + 1) * 128].rearrange(
                        "p t (u w) -> p t u w", u=16, w=8)
                    nc.vector.tensor_tensor(out=tmpA[:, :, q, 0:8, :],
                                            in0=Wq[:, :, 0:8, :], in1=s1s, op=Alu.mult)
                    nc.vector.tensor_tensor(out=tmpA[:, :, q, 8:16, :],
                                            in0=Wq[:, :, 8:16, :], in1=s1d, op=Alu.mult)
                Wvv = W1[:, :, 256:384].rearrange("p t (u w) -> p t u w", u=16, w=8)
                vdb = vdot1[:].unsqueeze(3).broadcast_to([P, t_w, 16, 8])
                nc.vector.tensor_tensor(out=tmpA[:, :, 2, :, :], in0=Wvv, in1=vdb, op=Alu.mult)
                tmpAm = tmpA[:].rearrange("p t q u w -> p t q (u w)")
                k = 8
                while k >= 1:
                    nc.vector.tensor_tensor(out=tmpAm[:, :, :, 0:k * 8],
                                            in0=tmpAm[:, :, :, 0:k * 8],
                                            in1=tmpAm[:, :, :, k * 8:2 * k * 8], op=Alu.add)
                    k //= 2
                st = sm.tile([P, t_w, 8], f32, tag="st")
                nc.vector.tensor_tensor(out=st[:], in0=tmpA[:, :, 0, 0, :],
                                        in1=tmpA[:, :, 2, 0, :], op=Alu.add)
                # S_sv (sqrt3-scaled) lives at tmpA[:, :, 1, 0, :]

                # ---- TP1 vs path: tmpB [p,t,16,8,3] --------------------------
                tmpB = big.tile([P, t_w, 16, 8, 3], f32, tag="tmpB")
                Wvs = W1[:, :, 384:512].rearrange("p t (u w) -> p t u w", u=16, w=8)
                for kk in range(3):
                    nc.vector.tensor_tensor(
                        out=tmpB[:, :, 0:8, :, kk],
                        in0=Wvs[:, :, 0:8, :],
                        in1=xsv[:, :, :, kk].unsqueeze(3).broadcast_to([P, t_w, 8, 8]),
                        op=Alu.mult)
                    nc.vector.tensor_tensor(
                        out=tmpB[:, :, 8:16, :, kk],
                        in0=Wvs[:, :, 8:16, :],
                        in1=xdv[:, :, :, kk].unsqueeze(3).broadcast_to([P, t_w, 8, 8]),
                        op=Alu.mult)
                tmpBm = tmpB[:].rearrange("p t u w k -> p t u (w k)")
                k = 8
                while k >= 1:
                    nc.vector.tensor_tensor(out=tmpBm[:, :, 0:k, :],
                                            in0=tmpBm[:, :, 0:k, :],
                                            in1=tmpBm[:, :, k:2 * k, :], op=Alu.add)
                    k //= 2

                # ---- v_t = S_sv*r + V_vs ; vdot2 = sum_k v_t r ---------------
                v_t = sm.tile([P, t_w, 8, 3], f32, tag="v_t")
                rbw = r[:].unsqueeze(2).broadcast_to([P, t_w, 8, 3])
                nc.vector.tensor_tensor(
                    out=v_t[:],
                    in0=tmpA[:, :, 1, 0, :].unsqueeze(3).broadcast_to([P, t_w, 8, 3]),
                    in1=rbw, op=Alu.mult)
                nc.vector.tensor_tensor(out=v_t[:], in0=v_t[:],
                                        in1=tmpB[:, :, 0, :, :], op=Alu.add)
                tmpD = sm.tile([P, t_w, 8, 3], f32, tag="tmpD")
                vdot2 = sm.tile([P, t_w, 8], f32, tag="vdot2")
                nc.vector.tensor_tensor(out=tmpD[:], in0=v_t[:], in1=rbw, op=Alu.mult)
                nc.vector.tensor_reduce(out=vdot2[:], in_=tmpD[:],
                                        axis=mybir.AxisListType.X, op=Alu.add)

                # ---- TP2 scalar paths ---------------------------------------
                tmpF = big.tile([P, t_w, 3, 8, 8], f32, tag="tmpF")
                tmpG = big.tile([P, t_w, 2, 8, 8], f32, tag="tmpE")
                stb = st[:].unsqueeze(3).broadcast_to([P, t_w, 8, 8])
                vd2b = vdot2[:].unsqueeze(3).broadcast_to([P, t_w, 8, 8])
                for q in range(3):
                    WFq = W2[:, :, q * 64:(q + 1) * 64].rearrange(
                        "p t (u w) -> p t u w", u=8, w=8)
                    nc.vector.tensor_tensor(out=tmpF[:, :, q, :, :], in0=WFq,
                                            in1=stb, op=Alu.mult)
                for q in range(2):
                    WGq = W2[:, :, 192 + q * 64:192 + (q + 1) * 64].rearrange(
                        "p t (u w) -> p t u w", u=8, w=8)
                    nc.vector.tensor_tensor(out=tmpG[:, :, q, :, :], in0=WGq,
                                            in1=vd2b, op=Alu.mult)
                tmpFm = tmpF[:].rearrange("p t q u w -> p t q (u w)")
                tmpGm = tmpG[:].rearrange("p t q u w -> p t q (u w)")
                k = 4
                while k >= 1:
                    nc.vector.tensor_tensor(out=tmpFm[:, :, :, 0:k * 8],
                                            in0=tmpFm[:, :, :, 0:k * 8],
                                            in1=tmpFm[:, :, :, k * 8:2 * k * 8], op=Alu.add)
                    nc.vector.tensor_tensor(out=tmpGm[:, :, :, 0:k * 8],
                                            in0=tmpGm[:, :, :, 0:k * 8],
                                            in1=tmpGm[:, :, :, k * 8:2 * k * 8], op=Alu.add)
                    k //= 2
                sg = sm.tile([P, t_w, 2, 8], f32, tag="sg")
                nc.vector.tensor_tensor(out=sg[:, :, 0, :], in0=tmpF[:, :, 0, 0, :],
                                        in1=tmpG[:, :, 0, 0, :], op=Alu.add)
                nc.vector.tensor_tensor(out=sg[:, :, 1, :], in0=tmpF[:, :, 1, 0, :],
                                        in1=tmpG[:, :, 1, 0, :], op=Alu.add)

                # ---- TP2 vs path (Cvs) --------------------------------------
                tmpH = big.tile([P, t_w, 8, 8, 3], f32, tag="tmpA")
                Wcvs = W2[:, :, 320:384].rearrange("p t (u w) -> p t u w", u=8, w=8)
                for kk in range(3):
                    nc.vector.tensor_tensor(
                        out=tmpH[:, :, :, :, kk],
                        in0=Wcvs,
                        in1=v_t[:, :, :, kk].unsqueeze(3).broadcast_to([P, t_w, 8, 8]),
                        op=Alu.mult)
                tmpHm = tmpH[:].rearrange("p t u w k -> p t u (w k)")
                k = 4
                while k >= 1:
                    nc.vector.tensor_tensor(out=tmpHm[:, :, 0:k, :],
                                            in0=tmpHm[:, :, 0:k, :],
                                            in1=tmpHm[:, :, k:2 * k, :], op=Alu.add)
                    k //= 2

                # ---- gate + norm + edge_out ---------------------------------
                tsg = sm.tile([P, t_w, 2, 8], f32, tag="tsg")
                nc.scalar.activation(tsg[:], sg[:], Act.Tanh)
                vecs = sm.tile([P, t_w, 8, 3], f32, tag="vecs")
                nc.vector.tensor_tensor(
                    out=vecs[:],
                    in0=tmpF[:, :, 2, 0, :].unsqueeze(3).broadcast_to([P, t_w, 8, 3]),
                    in1=rbw, op=Alu.mult)
                nc.vector.tensor_tensor(out=vecs[:], in0=vecs[:],
                                        in1=tmpH[:, :, 0, :, :], op=Alu.add)
                # tgn = tanh(gates)*norm (folds the norm scale of the vector
                # block into the gate); tsn = tanh(scal)*norm
                tgn = sm.tile([P, t_w, 2, 8], f32, tag="tgn")
                nc.vector.tensor_tensor(
                    out=tgn[:], in0=tsg[:],
                    in1=nrm[:].unsqueeze(2).unsqueeze(3).broadcast_to([P, t_w, 2, 8]),
                    op=Alu.mult)
                eo = sm.tile([P, t_w, 32], f32, tag="eo")
                nc.vector.tensor_copy(eo[:, :, 0:8], tgn[:, :, 0, :])
                nc.vector.tensor_tensor(
                    out=eo[:, :, 8:32].rearrange("p t (w k) -> p t w k", w=8, k=3),
                    in0=vecs[:],
                    in1=tgn[:, :, 1, :].unsqueeze(3).broadcast_to([P, t_w, 8, 3]),
                    op=Alu.mult)

                # ---- windowed segment-sum via accumulating matmul -----------
                oht = ohp.tile([P, t_w, P], f32, tag="oht")
                nc.vector.tensor_tensor(
                    out=oht[:],
                    in0=dstf[:].unsqueeze(2).broadcast_to([P, t_w, P]),
                    in1=iota_f[:].unsqueeze(1).broadcast_to([P, t_w, P]),
                    op=Alu.is_equal)
                acc = aps.tile([P, 32], f32, tag="acc")
                for t in range(t_w):
                    nc.tensor.matmul(out=acc[:], lhsT=oht[:, t, :], rhs=eo[:, t, :],
                                     start=(t == 0), stop=(t == t_w - 1),
                                     skip_group_check=True)
                osb = sm.tile([P, 32], f32, tag="osb")
                nc.scalar.copy(osb[:], acc[:])
                nc.sync.dma_start(out_d[w * P:(w + 1) * P, :], osb[:])
    nc.compile()
    return nc


def _get_nc(key):
    if key not in _KERNEL_CACHE:
        _KERNEL_CACHE[key] = _build(*key)
    return _KERNEL_CACHE[key]


# --------------------------------------------------------------------------
# Entry point
# --------------------------------------------------------------------------

def kernel(x, edge_src, edge_dst, edge_vec, emb, norm, num_nodes,
           fc1_w1, fc1_w2, fc2_w1, fc2_w2, _trace=False):
    x = np.asarray(x, np.float32)
    edge_src = np.asarray(edge_src).astype(np.int64)
    edge_dst = np.asarray(edge_dst).astype(np.int64)
    edge_vec = np.asarray(edge_vec, np.float32)
    emb = np.asarray(emb, np.float32)
    norm = np.asarray(norm, np.float32)
    fc1_w1 = np.asarray(fc1_w1, np.float32)
    fc1_w2 = np.asarray(fc1_w2, np.float32)
    fc2_w1 = np.asarray(fc2_w1, np.float32)
    fc2_w2 = np.asarray(fc2_w2, np.float32)
    N = x.shape[0]
    assert int(num_nodes) == N

    in_maps, N, npc, wpc, t_w, nt, ep = _host_prep(
        x, edge_src, edge_dst, edge_vec, emb, norm,
        fc1_w1, fc1_w2, fc2_w1, fc2_w2)
    nc = _get_nc((N, npc, wpc, t_w, nt, ep))
    res = run_bass_kernel_spmd(nc, in_maps, core_ids=list(range(NCORES)),
                               trace=_trace)
    out = np.concatenate([res.results[m]["out"] for m in range(NCORES)], axis=0)
    if _trace:
        return out[:N].astype(np.float32), res
    return out[:N].astype(np.float32)
